# revision 12
# baseline (speedup 1.0000x reference)
"""Trainium2 Bass kernel for nn_DeltaMemoryHead (scatter_memory).

Strategy (8 NeuronCores, data-parallel over batch B):
  - Each core handles B/8 = 4096 rows of x/target.
  - x is cast fp32->bf16 during the DMA load (SWDGE cast), then transposed
    on-chip via the HWDGE xbar DMA-transpose into [D-part, B-free] blocks.
  - One fused projection matmul per 128-row subtile computes
    [k | v | q | prec_pre] = x @ [Wk|Wv|Wv|Wprec] (+bias via a K=1 matmul),
    accumulating over 16 K-chunks in PSUM.  beta/tau pre-acts ride along as
    an N=2 matmul re-using the same stationary operand.
  - normalize() is folded into per-row scalars (rsqrt via Ln+Exp on ACT).
  - pred = k_norm @ S is accumulated with -S directly onto the v region of
    PSUM, so the tile holds u = v - pred;  err_u = 4/(1+e^u) - 2.
  - retrieved = q_norm @ S is accumulated into the dead q region.
  - Batch-sums (k_mean/err/prec/beta/tau) via ones-vector matmuls that
    accumulate into a persistent PSUM bank across all subtiles.
  - Per-core partial sums are returned to the host; the final tiny rank-1
    state update (S_new/momentum_new, 128x128) is computed on the host.
  - Only Exp/Ln/Copy/Identity ACT functions are used -> a single ACT table
    set (natural_log_exp_and_others), loaded once.
"""

import numpy as np
import ml_dtypes

B, D, DK, DV = 32768, 2048, 128, 128
NCORES = 8
BLOC = B // NCORES            # 4096 rows per core
NSUB = BLOC // 128            # 32 subtiles of 128 rows
NPAIR = NSUB // 2             # 16 pairs (2 subtiles share a PSUM tile)
KC = D // 128                 # 16 contraction chunks

_CACHE = {}


def _build_nc(npair=NPAIR, ablate=frozenset(), stage=99):
    from contextlib import ExitStack

    import concourse.bacc as bacc
    import concourse.tile as tile
    from concourse import mybir

    f32 = mybir.dt.float32
    bf16 = mybir.dt.bfloat16
    AF = mybir.ActivationFunctionType
    ALU = mybir.AluOpType

    nc = bacc.Bacc("TRN2", debug=False, target_bir_lowering=False,
                   num_devices=NCORES)

    x_d = nc.dram_tensor("x", [BLOC, D], f32, kind="ExternalInput").ap()
    tgt_d = nc.dram_tensor("target", [BLOC, DV], f32, kind="ExternalInput").ap()
    wmain_d = nc.dram_tensor("wmain", [128, KC * 512], bf16, kind="ExternalInput").ap()
    wbt_d = nc.dram_tensor("wbt", [128, KC * 2], bf16, kind="ExternalInput").ap()
    bmain_d = nc.dram_tensor("bmain", [1, 512], bf16, kind="ExternalInput").ap()
    bbt_d = nc.dram_tensor("bbt", [1, 2], bf16, kind="ExternalInput").ap()
    spos_d = nc.dram_tensor("spos", [DK, DV], bf16, kind="ExternalInput").ap()
    sneg_d = nc.dram_tensor("sneg", [DK, DV], bf16, kind="ExternalInput").ap()
    ident_d = nc.dram_tensor("ident", [128, 128], bf16, kind="ExternalInput").ap()

    f_d = nc.dram_tensor("f_out", [128, NSUB], f32, kind="ExternalOutput").ap()
    prec_d = nc.dram_tensor("prec_out", [BLOC, DV], f32, kind="ExternalOutput").ap()
    err_d = nc.dram_tensor("err_out", [BLOC, DV], f32, kind="ExternalOutput").ap()
    sums_d = nc.dram_tensor("sums_out", [1, 386], f32, kind="ExternalOutput").ap()

    with tile.TileContext(nc) as tc, ExitStack() as ctx:
        const = ctx.enter_context(tc.tile_pool(name="const", bufs=1))
        xin = ctx.enter_context(tc.tile_pool(name="xin", bufs=3))
        xtp = ctx.enter_context(tc.tile_pool(name="xtp", bufs=2))
        io = ctx.enter_context(tc.tile_pool(name="io", bufs=2))
        work = ctx.enter_context(tc.tile_pool(name="work", bufs=2))
        ps_main = ctx.enter_context(tc.tile_pool(name="ps_main", bufs=2, space="PSUM"))
        ps_tp = ctx.enter_context(tc.tile_pool(name="ps_tp", bufs=1, space="PSUM"))
        ps_bt = ctx.enter_context(tc.tile_pool(name="ps_bt", bufs=2, space="PSUM"))
        ps_sums = ctx.enter_context(tc.tile_pool(name="ps_sums", bufs=1, space="PSUM"))

        # ---- constants / weights ----
        w_sb = const.tile([128, KC * 512], bf16)
        nc.scalar.dma_start(out=w_sb[:, :], in_=wmain_d[:, :])
        wbt_sb = const.tile([128, KC * 2], bf16)
        nc.scalar.dma_start(out=wbt_sb[:, :], in_=wbt_d[:, :])
        bmain_sb = const.tile([1, 512], bf16)
        nc.scalar.dma_start(out=bmain_sb[:, :], in_=bmain_d[:, :])
        bbt_sb = const.tile([1, 2], bf16)
        nc.scalar.dma_start(out=bbt_sb[:, :], in_=bbt_d[:, :])
        spos_sb = const.tile([DK, DV], bf16)
        nc.scalar.dma_start(out=spos_sb[:, :], in_=spos_d[:, :])
        sneg_sb = const.tile([DK, DV], bf16)
        nc.scalar.dma_start(out=sneg_sb[:, :], in_=sneg_d[:, :])
        ident_sb = const.tile([128, 128], bf16)
        nc.scalar.dma_start(out=ident_sb[:, :], in_=ident_d[:, :])

        ones_row = const.tile([1, 128], bf16)
        nc.vector.memset(ones_row[:, :], 1.0)
        ones_bf = const.tile([128, 1], bf16)
        nc.vector.memset(ones_bf[:, :], 1.0)
        ones_f32 = const.tile([128, 1], f32)
        nc.vector.memset(ones_f32[:, :], 1.0)
        f_strip = None
        if stage > 5:
            f_strip = const.tile([128, NSUB], f32, name="f_strip")

        sums = None
        if stage > 5:
            sums = ps_sums.tile([1, 386], f32, name="sums")

        for t in range(npair):
            r0 = 256 * t
            # ---- load x (fp32 -> bf16 cast in DMA), one per subtile ----
            x_nat = xin.tile([128, 2, D], bf16, tag="xnat")
            for s in range(2):
                nc.gpsimd.dma_start(out=x_nat[:, s, :],
                                    in_=x_d[r0 + 128 * s: r0 + 128 * (s + 1), :])
            tgt = io.tile([128, 2, 128], f32, tag="tgt")
            if "rearr" in ablate:
                for s in range(2):
                    nc.scalar.dma_start(out=tgt[:, s, :],
                                        in_=tgt_d[r0 + 128 * s:r0 + 128 * (s + 1), :])
            else:
                nc.scalar.dma_start(
                    out=tgt[:, :, :],
                    in_=tgt_d[r0:r0 + 256, :].rearrange("(s p) d -> p s d", p=128))

            # ---- transpose x into [d-part, b-free] blocks (xbar DMA) ----
            xT = xtp.tile([128, 2, KC, 128], bf16, tag="xT")
            for s in range(2):
                for c in range(KC):
                    nc.sync.dma_start(out=xT[:, s, c, :],
                                      in_=x_nat[:, s, 128 * c:128 * (c + 1)],
                                      transpose=True)

            if stage <= 1:
                jk = work.tile([128, 2, 128], f32, tag="jk1")
                nc.vector.tensor_copy(jk[:, :, :], xT[:, :, 0, :])
                for s in range(2):
                    nc.scalar.dma_start(out=prec_d[r0 + 128 * s:r0 + 128 * (s + 1), :],
                                        in_=jk[:, s, :])
                continue
            # ---- projection matmuls ----
            main = ps_main.tile([128, 2, 512], f32, tag="main")
            btp = ps_bt.tile([128, 2, 2], f32, tag="btp")
            for c in range(KC):
                for s in range(2):
                    lhs = xT[:, s, c, :]
                    nc.tensor.matmul(main[:, s, :], lhs,
                                     w_sb[:, 512 * c:512 * (c + 1)],
                                     start=(c == 0), stop=False,
                                     skip_group_check=True)
                    if "bt" not in ablate:
                        nc.tensor.matmul(btp[:, s, :], lhs,
                                         wbt_sb[:, 2 * c:2 * (c + 1)],
                                         start=(c == 0), stop=False,
                                         skip_group_check=True)
            for s in range(2):
                nc.tensor.matmul(main[:, s, :], ones_row[:, :], bmain_sb[:, :],
                                 start=False, stop=True, skip_group_check=True)
                if "bt" not in ablate:
                    nc.tensor.matmul(btp[:, s, :], ones_row[:, :], bbt_sb[:, :],
                                     start=False, stop=True, skip_group_check=True)

            if stage <= 2:
                jk2 = work.tile([128, 2, 128], f32, tag="jk2")
                nc.scalar.activation(jk2[:, :, :], main[:, :, 384:512], AF.Copy)
                for s in range(2):
                    nc.scalar.dma_start(out=prec_d[r0 + 128 * s:r0 + 128 * (s + 1), :],
                                        in_=jk2[:, s, :])
                if "bt" not in ablate:
                    jk3 = work.tile([128, 2, 2], f32, tag="jk3")
                    nc.scalar.activation(jk3[:, :, :], btp[:, :, :], AF.Copy)
                    nc.scalar.dma_start(out=f_d[:, 2 * t:2 * t + 2],
                                        in_=jk3[:, 0, :])
                continue
            # ---- row norms of q_raw, k_raw (ACT Square + accum) ----
            qkss = work.tile([128, 4], f32, tag="qkss")  # [qA,kA,qB,kB]
            junk = work.tile([128, 2, 128], bf16, tag="junk")
            for s in range(2):
                nc.scalar.activation(junk[:, s, :], main[:, s, 256:384],
                                     AF.Square,
                                     accum_out=qkss[:, 2 * s:2 * s + 1])
                nc.scalar.activation(junk[:, s, :], main[:, s, 0:128],
                                     AF.Square,
                                     accum_out=qkss[:, 2 * s + 1:2 * s + 2])
            # 1/sqrt(ss) = exp(-0.5*ln(ss))
            qkln = work.tile([128, 4], f32, tag="qkln")
            nc.scalar.activation(qkln[:, :], qkss[:, :], AF.Ln)
            qkinv = work.tile([128, 4], f32, tag="qkinv")
            nc.scalar.activation(qkinv[:, :], qkln[:, :], AF.Exp, scale=-0.5)

            qn = work.tile([128, 2, 128], bf16, tag="qn")
            kn = work.tile([128, 2, 128], bf16, tag="kn")
            for s in range(2):
                nc.vector.tensor_scalar(
                    out=qn[:, s, :], in0=main[:, s, 256:384],
                    scalar1=qkinv[:, 2 * s:2 * s + 1], scalar2=None,
                    op0=ALU.mult)
                nc.vector.tensor_scalar(
                    out=kn[:, s, :], in0=main[:, s, 0:128],
                    scalar1=qkinv[:, 2 * s + 1:2 * s + 2], scalar2=None,
                    op0=ALU.mult)

            if stage <= 3:
                jk4 = work.tile([128, 2, 128], f32, tag="jk4")
                nc.vector.tensor_copy(jk4[:, :, :], qn[:, :, :])
                for s in range(2):
                    nc.scalar.dma_start(out=prec_d[r0 + 128 * s:r0 + 128 * (s + 1), :],
                                        in_=jk4[:, s, :])
                jk5 = work.tile([128, 2, 128], f32, tag="jk5")
                nc.scalar.activation(jk5[:, :, :], main[:, :, 384:512], AF.Copy)
                for s in range(2):
                    nc.scalar.dma_start(out=err_d[r0 + 128 * s:r0 + 128 * (s + 1), :],
                                        in_=jk5[:, s, :])
                continue
            # ---- transpose q_norm / k_norm on PE ----
            if "qk" not in ablate:
                tp = ps_tp.tile([128, 4, 128], bf16, tag="tp")  # qtA,ktA,qtB,ktB
                qtkt = work.tile([128, 4, 128], bf16, tag="qtkt")
                for s in range(2):
                    nc.tensor.transpose(tp[:, 2 * s, :], qn[:, s, :], ident_sb[:, :])
                    nc.tensor.transpose(tp[:, 2 * s + 1, :], kn[:, s, :], ident_sb[:, :])
                for s in range(2):
                    nc.scalar.copy(qtkt[:, 2 * s, :], tp[:, 2 * s, :])
                    nc.vector.tensor_copy(qtkt[:, 2 * s + 1, :], tp[:, 2 * s + 1, :])

                # ---- pred onto v region (u = v - k_norm@S); then retrieved ----
                for s in range(2):
                    nc.tensor.matmul(main[:, s, 128:256], qtkt[:, 2 * s + 1, :],
                                     sneg_sb[:, :], start=False, stop=True,
                                     skip_group_check=True)
                for s in range(2):
                    nc.tensor.matmul(main[:, s, 256:384], qtkt[:, 2 * s, :],
                                     spos_sb[:, :], start=True, stop=True,
                                     skip_group_check=True)

            if stage <= 4:
                jk6 = work.tile([128, 2, 128], f32, tag="jk6")
                nc.scalar.activation(jk6[:, :, :], main[:, :, 256:384], AF.Copy)
                for s in range(2):
                    nc.scalar.dma_start(out=prec_d[r0 + 128 * s:r0 + 128 * (s + 1), :],
                                        in_=jk6[:, s, :])
                jk7 = work.tile([128, 2, 128], f32, tag="jk7")
                nc.vector.tensor_sub(jk7[:, 0, :], main[:, 0, 128:256], tgt[:, 0, :])
                nc.vector.tensor_sub(jk7[:, 1, :], main[:, 1, 128:256], tgt[:, 1, :])
                for s in range(2):
                    nc.scalar.dma_start(out=err_d[r0 + 128 * s:r0 + 128 * (s + 1), :],
                                        in_=jk7[:, s, :])
                continue
            # ---- prec = softplus(pre) + 0.01 ----
            p1 = work.tile([128, 2, 128], f32, tag="p1")
            nc.scalar.activation(p1[:, :, :], main[:, :, 384:512], AF.Exp)
            spf = work.tile([128, 2, 128], f32, tag="spf")
            nc.scalar.activation(spf[:, :, :], p1[:, :, :], AF.Ln, bias=1.0)
            prec_f = io.tile([128, 2, 128], f32, tag="precf")
            nc.vector.tensor_scalar(out=prec_f[:, :, :], in0=spf[:, :, :],
                                    scalar1=0.01, scalar2=None, op0=ALU.add)
            if "rearr" in ablate:
                for s in range(2):
                    nc.scalar.dma_start(out=prec_d[r0 + 128 * s:r0 + 128 * (s + 1), :],
                                        in_=prec_f[:, s, :])
            else:
                nc.scalar.dma_start(
                    out=prec_d[r0:r0 + 256, :].rearrange("(s p) d -> p s d", p=128),
                    in_=prec_f[:, :, :])

            # ---- F pieces ----
            fab = work.tile([128, 4], f32, tag="fab")  # [FaA,FbA,FaB,FbB]
            lpj = work.tile([128, 2, 128], bf16, tag="lpj")
            for s in range(2):
                nc.scalar.activation(lpj[:, s, :], prec_f[:, s, :], AF.Ln,
                                     accum_out=fab[:, 2 * s + 1:2 * s + 2])

            if stage <= 5:
                jk8 = work.tile([128, 2, 128], f32, tag="jk8")
                nc.scalar.activation(jk8[:, :, :], main[:, :, 128:256], AF.Copy)
                for s in range(2):
                    nc.scalar.dma_start(out=err_d[r0 + 128 * s:r0 + 128 * (s + 1), :],
                                        in_=jk8[:, s, :])
                continue
            # ---- R = 1/(1+e^u) ----
            ee = work.tile([128, 2, 128], f32, tag="ee")
            nc.scalar.activation(ee[:, :, :], main[:, :, 128:256], AF.Exp)
            aa = work.tile([128, 2, 128], f32, tag="aa")
            nc.vector.tensor_scalar(out=aa[:, :, :], in0=ee[:, :, :],
                                    scalar1=1.0, scalar2=None, op0=ALU.add)
            rr = work.tile([128, 2, 128], f32, tag="rr")
            if "recip" in ablate:
                nc.vector.tensor_copy(rr[:, :, :], aa[:, :, :])
            else:
                nc.vector.reciprocal(rr[:, :, :], aa[:, :, :])
            rrb = work.tile([128, 2, 128], bf16, tag="rrb")
            nc.vector.tensor_copy(rrb[:, :, :], rr[:, :, :])

            # ---- error = target - retrieved ----
            err = io.tile([128, 2, 128], f32, tag="err")
            for s in range(2):
                nc.vector.tensor_sub(err[:, s, :], tgt[:, s, :],
                                     main[:, s, 256:384])
            if "rearr" in ablate:
                for s in range(2):
                    nc.scalar.dma_start(out=err_d[r0 + 128 * s:r0 + 128 * (s + 1), :],
                                        in_=err[:, s, :])
            else:
                nc.scalar.dma_start(
                    out=err_d[r0:r0 + 256, :].rearrange("(s p) d -> p s d", p=128),
                    in_=err[:, :, :])
            e2 = work.tile([128, 2, 128], f32, tag="e2")
            nc.vector.tensor_mul(e2[:, :, :], err[:, :, :], err[:, :, :])
            pe2 = work.tile([128, 2, 128], f32, tag="pe2")
            nc.vector.tensor_mul(pe2[:, :, :], e2[:, :, :], prec_f[:, :, :])
            for s in range(2):
                nc.vector.tensor_reduce(
                    out=fab[:, 2 * s:2 * s + 1], in_=pe2[:, s, :],
                    axis=mybir.AxisListType.X, op=ALU.add)
            for s in range(2):
                if "fcomb" in ablate:
                    nc.vector.tensor_copy(f_strip[:, 2 * t + s:2 * t + s + 1],
                                          fab[:, 2 * s:2 * s + 1])
                else:
                    nc.vector.tensor_scalar(
                        out=f_strip[:, 2 * t + s:2 * t + s + 1],
                        in0=fab[:, 2 * s:2 * s + 1],
                        scalar1=fab[:, 2 * s + 1:2 * s + 2],
                        scalar2=1.0 / 128.0,
                        op0=ALU.subtract, op1=ALU.mult)

            # ---- beta/tau sigmoids ----
            if "bt" in ablate:
                rbt = work.tile([128, 2, 2], f32, tag="rbt")
                nc.vector.memset(rbt[:, :, :], 0.5)
            else:
                ebt = work.tile([128, 2, 2], f32, tag="ebt")
                nc.scalar.activation(ebt[:, :, :], btp[:, :, :], AF.Exp, scale=-1.0)
                abt = work.tile([128, 2, 2], f32, tag="abt")
                nc.vector.tensor_scalar(out=abt[:, :, :], in0=ebt[:, :, :],
                                        scalar1=1.0, scalar2=None, op0=ALU.add)
                rbt = work.tile([128, 2, 2], f32, tag="rbt")
                if "recip" in ablate:
                    nc.vector.tensor_copy(rbt[:, :, :], abt[:, :, :])
                else:
                    nc.vector.reciprocal(rbt[:, :, :], abt[:, :, :])

            # ---- batch sums (accumulate into persistent PSUM bank) ----
            if "sums" in ablate:
                continue
            first = (t == 0)
            last = (t == npair - 1)
            for s in range(2):
                nc.tensor.matmul(sums[:, 0:128], ones_bf[:, :], kn[:, s, :],
                                 start=(first and s == 0), stop=False,
                                 skip_group_check=True)
                nc.tensor.matmul(sums[:, 128:256], ones_bf[:, :], rrb[:, s, :],
                                 start=False, stop=False, skip_group_check=True)
                nc.tensor.matmul(sums[:, 256:384], ones_f32[:, :],
                                 prec_f[:, s, :], start=False, stop=False,
                                 skip_group_check=True)
                nc.tensor.matmul(sums[:, 384:386], ones_f32[:, :],
                                 rbt[:, s, :], start=False,
                                 stop=(last and s == 1), skip_group_check=True)

        # ---- tail: flush sums + F ----
        if "sums" not in ablate and stage > 5:
            sums_sb = const.tile([1, 386], f32)
            nc.vector.tensor_copy(sums_sb[:, :], sums[:, :])
            nc.scalar.dma_start(out=sums_d[:, :], in_=sums_sb[:, :])
        if stage > 5:
            nc.scalar.dma_start(out=f_d[:, :], in_=f_strip[:, :])

    nc.compile()
    return nc


def _get_nc():
    if "nc" not in _CACHE:
        _CACHE["nc"] = _build_nc()
    return _CACHE["nc"]


def _prep_weights(Wk, bk, Wv, bv, Wq, bq, Wbeta, bbeta, Wprec, bprec,
                  Wtau, btau, S):
    bf16 = ml_dtypes.bfloat16
    wmain = np.concatenate([Wk, Wv, Wq, Wprec], axis=1)        # [D, 512]
    wmain = np.ascontiguousarray(
        wmain.reshape(KC, 128, 512).transpose(1, 0, 2).reshape(128, KC * 512)
    ).astype(bf16)
    wbt = np.concatenate([Wbeta, Wtau], axis=1)                # [D, 2]
    wbt = np.ascontiguousarray(
        wbt.reshape(KC, 128, 2).transpose(1, 0, 2).reshape(128, KC * 2)
    ).astype(bf16)
    bmain = np.concatenate([bk, bv, bq, bprec]).reshape(1, 512).astype(bf16)
    bbt = np.array([[bbeta[0], btau[0]]], dtype=np.float32).astype(bf16)
    spos = S.astype(bf16)
    sneg = (-S).astype(bf16)
    ident = np.eye(128, dtype=np.float32).astype(bf16)
    return dict(wmain=wmain, wbt=wbt, bmain=bmain, bbt=bbt, spos=spos,
                sneg=sneg, ident=ident)


def kernel(x, target, Wk, bk, Wv, bv, Wq, bq, Wbeta, bbeta, Wprec, bprec,
           Wtau, btau, S, momentum):
    from concourse.bass_utils import run_bass_kernel_spmd

    x = np.asarray(x, dtype=np.float32)
    target = np.asarray(target, dtype=np.float32)
    args = [np.asarray(a, dtype=np.float32) for a in
            (Wk, bk, Wv, bv, Wq, bq, Wbeta, bbeta, Wprec, bprec, Wtau, btau, S)]
    momentum = np.asarray(momentum, dtype=np.float32)
    S = args[-1]

    nc = _get_nc()
    wd = _prep_weights(*args)
    in_maps = []
    for i in range(NCORES):
        m = dict(wd)
        m["x"] = x[i * BLOC:(i + 1) * BLOC]
        m["target"] = target[i * BLOC:(i + 1) * BLOC]
        in_maps.append(m)

    res = run_bass_kernel_spmd(nc, in_maps, core_ids=list(range(NCORES)))
    _CACHE["last_res"] = res
    rs = res.results

    F_mean = np.concatenate([r["f_out"].T.reshape(-1) for r in rs])
    prec = np.concatenate([r["prec_out"] for r in rs], axis=0)
    error = np.concatenate([r["err_out"] for r in rs], axis=0)
    sums = np.sum(np.stack([r["sums_out"][0] for r in rs]), axis=0,
                  dtype=np.float64)

    Bf = np.float64(B)
    k_mean = (sums[0:128] / Bf).astype(np.float32)
    error_mean = (4.0 * sums[128:256] / Bf - 2.0).astype(np.float32)
    prec_mean = (sums[256:384] / Bf).astype(np.float32)
    beta_mean = np.float32(sums[384] / Bf)
    tau_mean = np.float32(1.0 + 19.0 * (sums[385] / Bf))

    decay = np.float32(1.0 / (1.0 + np.exp(0.1)) + 0.5)
    effective_decay = np.float32(1.0) - (np.float32(1.0) - decay) / tau_mean
    surprise = beta_mean * prec_mean * error_mean                  # (DV,)
    upd = np.outer(k_mean, surprise).astype(np.float32)            # (DK, DV)
    momentum_new = 0.9 * momentum + 0.1 * upd
    S_new = effective_decay * S - 0.01 * momentum_new

    return (F_mean, prec, error,
            S_new.astype(np.float32), momentum_new.astype(np.float32))
